# revision 4
# baseline (speedup 1.0000x reference)
import sys
sys.path.insert(0, "/opt/trn_rl_repo")
import time
import numpy as np
import ml_dtypes

N_NODES = 131072
N_EDGES = 2097152
N_GRAPHS = 2048
IN_CH, HID, OUT = 12, 64, 4
NCORES = 8
COLS = 132
NL = COLS * 128          # 16896 padded nodes per core
NTOT = NCORES * NL       # padded global table rows
G_PAD = 512              # graph slots per core (padded)

_cache = {}
LAST_EXEC_WALLS = []
LAST_EXEC_NS = []


def _build(K_cols):
    """Single-launch SPMD kernel. K_cols[c] = neighbor slots in column c
    (shared across cores). Each column also has 1 self slot."""
    import concourse.bass as bass
    import concourse.bacc as bacc
    import concourse.tile as tile
    import concourse.mybir as mybir
    dt = mybir.dt
    AF = mybir.ActivationFunctionType

    S = sum(k + 1 for k in K_cols)
    NCHUNK = NL // 512     # 33 chunks of 512 nodes (4 columns each)

    nc = bacc.Bacc("TRN2", target_bir_lowering=False, debug=False,
                   num_devices=NCORES)
    xp_in = nc.dram_tensor("xp", [128, COLS, IN_CH], dt.bfloat16,
                           kind="ExternalInput").ap()
    idx_in = nc.dram_tensor("idx", [128, S], dt.int32, kind="ExternalInput").ap()
    dinv_in = nc.dram_tensor("dinv", [128, COLS], dt.float32,
                             kind="ExternalInput").ap()
    gid_in = nc.dram_tensor("gid", [128, COLS], dt.float32,
                            kind="ExternalInput").ap()
    cnt_in = nc.dram_tensor("cntinv", [OUT, G_PAD], dt.float32,
                            kind="ExternalInput").ap()
    w1_in = nc.dram_tensor("W1", [IN_CH, HID], dt.float32, kind="ExternalInput").ap()
    b1_in = nc.dram_tensor("b1", [HID, 1], dt.float32, kind="ExternalInput").ap()
    w2_in = nc.dram_tensor("W2", [HID, HID], dt.float32, kind="ExternalInput").ap()
    b2b_in = nc.dram_tensor("b2b", [128, HID], dt.float32, kind="ExternalInput").ap()
    wfc_in = nc.dram_tensor("Wfc", [HID, OUT], dt.float32, kind="ExternalInput").ap()
    bfc_in = nc.dram_tensor("bfc", [OUT, 1], dt.float32, kind="ExternalInput").ap()
    outp = nc.dram_tensor("outp", [OUT, G_PAD], dt.float32,
                          kind="ExternalOutput").ap()

    t1_local = nc.dram_tensor("t1_local", [NL, IN_CH], dt.bfloat16)
    T1 = nc.dram_tensor("T1", [NTOT, IN_CH], dt.bfloat16)
    t2_local = nc.dram_tensor("t2_local", [NL, HID], dt.bfloat16)
    T2 = nc.dram_tensor("T2", [NTOT, HID], dt.bfloat16)

    I128 = nc.inline_tensor(np.eye(128, dtype=np.float32), name="I128").ap()
    I64 = nc.inline_tensor(np.eye(HID, dtype=np.float32), name="I64").ap()

    rg = [list(range(NCORES))]

    with tile.TileContext(nc) as tc:
        with (
            tc.tile_pool(name="p", bufs=1) as pool,
            tc.tile_pool(name="pt", bufs=2) as tpool,
            tc.tile_pool(name="psA", bufs=2, space=bass.MemorySpace.PSUM) as psA,
            tc.tile_pool(name="psB", bufs=1, space=bass.MemorySpace.PSUM) as psB,
            tc.tile_pool(name="psP", bufs=1, space=bass.MemorySpace.PSUM) as psP,
        ):
            # ---- load inputs ----
            xp_t = pool.tile([128, COLS, IN_CH], dt.bfloat16, name="xp_t")
            nc.sync.dma_start(xp_t[:], xp_in[:])
            idx_t = pool.tile([128, S], dt.int32, name="idx_t")
            nc.sync.dma_start(idx_t[:], idx_in[:])
            dinv_t = pool.tile([128, COLS], dt.float32, name="dinv_t")
            nc.sync.dma_start(dinv_t[:], dinv_in[:])
            gid_t = pool.tile([128, COLS], dt.float32, name="gid_t")
            nc.sync.dma_start(gid_t[:], gid_in[:])
            cnt_t = pool.tile([OUT, G_PAD], dt.float32, name="cnt_t")
            nc.sync.dma_start(cnt_t[:], cnt_in[:])
            w1_t = pool.tile([IN_CH, HID], dt.float32, name="w1_t")
            nc.sync.dma_start(w1_t[:], w1_in[:])
            b1_t = pool.tile([HID, 1], dt.float32, name="b1_t")
            nc.sync.dma_start(b1_t[:], b1_in[:])
            w2_t = pool.tile([HID, HID], dt.float32, name="w2_t")
            nc.sync.dma_start(w2_t[:], w2_in[:])
            b2b_t = pool.tile([128, HID], dt.float32, name="b2b_t")
            nc.sync.dma_start(b2b_t[:], b2b_in[:])
            wfc_t = pool.tile([HID, OUT], dt.float32, name="wfc_t")
            nc.sync.dma_start(wfc_t[:], wfc_in[:])
            bfc_t = pool.tile([OUT, 1], dt.float32, name="bfc_t")
            nc.sync.dma_start(bfc_t[:], bfc_in[:])
            i128_t = pool.tile([128, 128], dt.float32, name="i128_t")
            nc.sync.dma_start(i128_t[:], I128[:])
            i64_t = pool.tile([HID, HID], dt.float32, name="i64_t")
            nc.sync.dma_start(i64_t[:], I64[:])

            iota_i = pool.tile([128, G_PAD], dt.int32, name="iota_i")
            nc.gpsimd.iota(iota_i[:], pattern=[[1, G_PAD]], base=0,
                           channel_multiplier=0)
            iota_f = pool.tile([128, G_PAD], dt.float32, name="iota_f")
            nc.vector.tensor_copy(iota_f[:], iota_i[:])

            # ---- t1 local block + AllGather ----
            nc.sync.dma_start(
                t1_local.ap().rearrange("(c p) f -> p c f", p=128), xp_t[:])
            nc.gpsimd.collective_compute(
                "AllGather", mybir.AluOpType.bypass, replica_groups=rg,
                ins=[t1_local.ap().opt()], outs=[T1.ap().opt()])

            # ---- L1 aggregation: per column K+1 indirect adds ----
            agg1 = pool.tile([128, COLS, IN_CH], dt.float32, name="agg1")
            A1 = [pool.tile([128, IN_CH], dt.float32, name=f"A1_{c}")
                  for c in range(COLS)]
            off = 0
            for c in range(COLS):
                nc.vector.memset(A1[c][:], 0.0)
                for j in range(K_cols[c] + 1):
                    nc.gpsimd.indirect_dma_start(
                        out=A1[c][:, :], out_offset=None, in_=T1.ap(),
                        in_offset=bass.IndirectOffsetOnAxis(
                            ap=idx_t[:, off + j:off + j + 1], axis=0),
                        compute_op=mybir.AluOpType.add)
                off += K_cols[c] + 1
                nc.scalar.mul(agg1[:, c, :], A1[c][:], dinv_t[:, c:c + 1])

            # ---- transpose agg1 -> rT [IN_CH, NL] ----
            rT = pool.tile([IN_CH, NL], dt.float32, name="rT")
            for c in range(COLS):
                pT = psA.tile([IN_CH, 128], dt.float32, name=f"pT{c}", tag="pT")
                nc.tensor.transpose(pT[:], agg1[:, c, :], i128_t[:])
                nc.scalar.copy(rT[:, c * 128:(c + 1) * 128], pT[:])

            # ---- per 512-node chunk: W1+relu+W2, transpose back, scale -> t2 ----
            for k in range(NCHUNK):
                sl = slice(k * 512, (k + 1) * 512)
                pW1 = psB.tile([HID, 512], dt.float32, name="pW1", tag="pW1")
                nc.tensor.matmul(pW1[:], w1_t[:], rT[:, sl])
                h1c = tpool.tile([HID, 512], dt.float32, name="h1c", tag="h1c")
                nc.scalar.activation(h1c[:], pW1[:], AF.Relu, bias=b1_t[:, 0:1])
                pW2 = psB.tile([HID, 512], dt.float32, name="pW2", tag="pW2")
                nc.tensor.matmul(pW2[:], w2_t[:], h1c[:])
                t2c = tpool.tile([HID, 512], dt.float32, name="t2c", tag="t2c")
                nc.scalar.copy(t2c[:], pW2[:])
                rows = tpool.tile([128, 4, HID], dt.bfloat16, name="rows", tag="rows")
                for q in range(4):
                    c = k * 4 + q
                    pR = psA.tile([128, HID], dt.float32, name=f"pR{c}", tag="pR")
                    nc.tensor.transpose(pR[:], t2c[:, q * 128:(q + 1) * 128], i64_t[:])
                    nc.scalar.mul(rows[:, q, :], pR[:], dinv_t[:, c:c + 1])
                nc.sync.dma_start(
                    t2_local.ap()[k * 512:(k + 1) * 512, :]
                    .rearrange("(q p) f -> p q f", p=128), rows[:])

            # ---- AllGather t2 ----
            nc.gpsimd.collective_compute(
                "AllGather", mybir.AluOpType.bypass, replica_groups=rg,
                ins=[t2_local.ap().opt()], outs=[T2.ap().opt()])

            # ---- L2 aggregation + h2 + pooling ----
            pPool = psP.tile([HID, G_PAD], dt.float32, name="pPool")
            off = 0
            for c in range(COLS):
                A2 = pool.tile([128, HID], dt.float32, name=f"A2_{c}")
                nc.vector.memset(A2[:], 0.0)
                for j in range(K_cols[c] + 1):
                    nc.gpsimd.indirect_dma_start(
                        out=A2[:, :], out_offset=None, in_=T2.ap(),
                        in_offset=bass.IndirectOffsetOnAxis(
                            ap=idx_t[:, off + j:off + j + 1], axis=0),
                        compute_op=mybir.AluOpType.add)
                off += K_cols[c] + 1
                h2a = tpool.tile([128, HID], dt.float32, name="h2a", tag="h2a")
                nc.scalar.mul(h2a[:], A2[:], dinv_t[:, c:c + 1])
                h2b = tpool.tile([128, HID], dt.float32, name="h2b", tag="h2b")
                nc.vector.tensor_add(h2b[:], h2a[:], b2b_t[:])
                h2col = tpool.tile([128, HID], dt.bfloat16, name="h2col", tag="h2col")
                nc.scalar.activation(h2col[:], h2b[:], AF.Relu)
                Pcol = tpool.tile([128, G_PAD], dt.bfloat16, name="Pcol", tag="Pcol")
                nc.vector.tensor_scalar(
                    Pcol[:], iota_f[:], gid_t[:, c:c + 1], None,
                    op0=mybir.AluOpType.is_equal)
                nc.tensor.matmul(pPool[:], h2col[:], Pcol[:],
                                 start=(c == 0), stop=(c == COLS - 1))

            # ---- FC + sigmoid ----
            pooledT = pool.tile([HID, G_PAD], dt.float32, name="pooledT")
            nc.scalar.copy(pooledT[:], pPool[:])
            pF = psB.tile([OUT, G_PAD], dt.float32, name="pF", tag="pF")
            nc.tensor.matmul(pF[:], wfc_t[:], pooledT[:])
            gsc = pool.tile([OUT, G_PAD], dt.float32, name="gsc")
            nc.vector.tensor_mul(gsc[:], pF[:], cnt_t[:])
            res = pool.tile([OUT, G_PAD], dt.float32, name="res")
            nc.scalar.activation(res[:], gsc[:], AF.Sigmoid, bias=bfc_t[:, 0:1])
            nc.sync.dma_start(outp[:], res[:])

    nc.compile()
    return nc


def kernel(x, edge_index, batch, W1, b1, W2, b2, Wfc, bfc):
    import os
    x = np.asarray(x, np.float32)
    src = np.asarray(edge_index[0]).astype(np.int64)
    dst = np.asarray(edge_index[1]).astype(np.int64)
    batch = np.asarray(batch).astype(np.int64)
    W1 = np.asarray(W1, np.float32); b1 = np.asarray(b1, np.float32)
    W2 = np.asarray(W2, np.float32); b2 = np.asarray(b2, np.float32)
    Wfc = np.asarray(Wfc, np.float32); bfc = np.asarray(bfc, np.float32)

    # ---------- host index preprocessing (vectorized) ----------
    deg = np.bincount(dst, minlength=N_NODES).astype(np.float32) + 1.0
    dinv = 1.0 / np.sqrt(deg)

    order = np.argsort(dst, kind="stable")
    dst_s = dst[order]; src_s = src[order]
    starts = np.searchsorted(dst_s, np.arange(N_NODES + 1))

    gcnt = np.bincount(batch, minlength=N_GRAPHS)
    gcum = np.concatenate([[0], np.cumsum(gcnt)])
    bounds = [0]
    gb = [0]
    for d in range(1, NCORES):
        tgt = d * (N_NODES // NCORES)
        g = int(np.argmin(np.abs(gcum - tgt)))
        bounds.append(int(gcum[g])); gb.append(g)
    bounds.append(N_NODES); gb.append(N_GRAPHS)

    ldeg_all = (starts[1:] - starts[:-1]).astype(np.int64)

    rank_of_node = np.empty(N_NODES, np.int64)   # node -> rank within owner
    owner_of_node = np.empty(N_NODES, np.int64)
    gn_of_rank = []                              # per core: rank -> node (-1 pad)
    colmax = np.zeros((NCORES, COLS), np.int64)
    for d in range(NCORES):
        s_d, e_d = bounds[d], bounds[d + 1]
        nloc = e_d - s_d
        assert nloc < NL, (nloc, NL)
        r2l = np.argsort(-ldeg_all[s_d:e_d], kind="stable")
        rank_of_node[s_d + r2l] = np.arange(nloc)
        owner_of_node[s_d:e_d] = d
        gn = np.full(NL, -1, np.int64)
        gn[:nloc] = s_d + r2l
        gn_of_rank.append(gn)
        rd = np.zeros(NL, np.int64)
        rd[:nloc] = ldeg_all[s_d:e_d][r2l]
        colmax[d] = rd.reshape(COLS, 128).max(axis=1)
    K_cols = tuple(int(v) for v in colmax.max(axis=0))
    S = sum(k + 1 for k in K_cols)
    soff = np.zeros(COLS, np.int64)              # slot offset per column
    acc = 0
    for c in range(COLS):
        soff[c] = acc
        acc += K_cols[c] + 1

    tablerow = owner_of_node * NL + rank_of_node     # node -> table row
    nlocs = np.array([bounds[d + 1] - bounds[d] for d in range(NCORES)])
    dmin = int(np.argmin(nlocs))
    PADROW = dmin * NL + NL - 1

    # per-core inputs
    in_maps = []
    for d in range(NCORES):
        s_d, e_d = bounds[d], bounds[d + 1]
        nloc = e_d - s_d
        gn = gn_of_rank[d]
        ok = gn >= 0

        ia = np.full((128, S), PADROW, np.int32)
        # self slots
        rr = np.arange(NL)
        pp = rr % 128; cc = rr // 128
        ia[pp[ok], soff[cc[ok]]] = (d * NL + rr[ok]).astype(np.int32)
        # neighbor slots (edge-driven, vectorized)
        e0, e1 = starts[s_d], starts[e_d]
        dst_e = dst_s[e0:e1]
        src_e = src_s[e0:e1]
        j_e = np.arange(e0, e1) - starts[dst_e]
        r_e = rank_of_node[dst_e]
        p_e = r_e % 128; c_e = r_e // 128
        ia[p_e, soff[c_e] + 1 + j_e] = tablerow[src_e].astype(np.int32)

        xp = np.zeros((128, COLS, IN_CH), np.float32)
        xv = x[gn[ok]] * dinv[gn[ok]][:, None]
        xp[pp[ok], cc[ok]] = xv

        dv = np.zeros((128, COLS), np.float32)
        dv[pp[ok], cc[ok]] = dinv[gn[ok]]

        ng = gb[d + 1] - gb[d]
        assert ng < G_PAD, ng
        gi = np.full((128, COLS), G_PAD - 1, np.float32)
        gi[pp[ok], cc[ok]] = (batch[gn[ok]] - gb[d]).astype(np.float32)

        ci = np.ones((OUT, G_PAD), np.float32)
        ci[:, :ng] = 1.0 / np.maximum(gcnt[gb[d]:gb[d + 1]], 1.0)[None, :]

        in_maps.append({
            "xp": xp.astype(ml_dtypes.bfloat16),
            "idx": ia,
            "dinv": dv,
            "gid": gi,
            "cntinv": ci,
            "W1": W1, "b1": b1.reshape(HID, 1), "W2": W2,
            "b2b": np.broadcast_to(b2, (128, HID)).copy(),
            "Wfc": Wfc, "bfc": bfc.reshape(OUT, 1),
        })

    # ---------- build + run ----------
    key = ("V2", K_cols)
    if key not in _cache:
        _cache[key] = _build(list(K_cols))
    nc = _cache[key]

    from concourse.bass_utils import run_bass_kernel_spmd
    trace = bool(os.environ.get("BASS_TRACE"))
    t0 = time.perf_counter()
    res = run_bass_kernel_spmd(nc, in_maps, core_ids=list(range(NCORES)),
                               trace=trace)
    LAST_EXEC_WALLS.append(time.perf_counter() - t0)
    if res.exec_time_ns is not None:
        LAST_EXEC_NS.append(res.exec_time_ns)

    out = np.zeros((N_GRAPHS, OUT), np.float32)
    for d in range(NCORES):
        ng = gb[d + 1] - gb[d]
        out[gb[d]:gb[d + 1]] = np.asarray(
            res.results[d]["outp"], np.float32)[:, :ng].T
    return out


# revision 12
# speedup vs baseline: 1.1811x; 1.1811x over previous
import sys
sys.path.insert(0, "/opt/trn_rl_repo")
import time
import numpy as np
import ml_dtypes

N_NODES = 131072
N_EDGES = 2097152
N_GRAPHS = 2048
IN_CH, HID, OUT = 12, 64, 4
NCORES = 8
COLS = 132
NL = COLS * 128          # 16896 padded nodes per core
NTOT = NCORES * NL       # padded global table rows
G_PAD = 512              # graph slots per core (padded)

_cache = {}
LAST_EXEC_WALLS = []
LAST_EXEC_NS = []


def _build(K_cols):
    """Single-launch SPMD kernel. K_cols[c] = neighbor slots in column c
    (shared across cores). Self term is added from SBUF, not gathered."""
    import concourse.bass as bass
    import concourse.bacc as bacc
    import concourse.tile as tile
    import concourse.mybir as mybir
    dt = mybir.dt
    AF = mybir.ActivationFunctionType

    S = sum(K_cols)
    NCHUNK = NL // 512     # 33 chunks of 512 nodes (4 columns each)

    nc = bacc.Bacc("TRN2", target_bir_lowering=False, debug=False,
                   num_devices=NCORES)
    xp_in = nc.dram_tensor("xp", [128, COLS, IN_CH], dt.bfloat16,
                           kind="ExternalInput").ap()
    idx_in = nc.dram_tensor("idx", [128, S], dt.int32, kind="ExternalInput").ap()
    dinv_in = nc.dram_tensor("dinv", [128, COLS], dt.float32,
                             kind="ExternalInput").ap()
    gid_in = nc.dram_tensor("gid", [128, COLS], dt.float32,
                            kind="ExternalInput").ap()
    cnt_in = nc.dram_tensor("cntinv", [OUT, G_PAD], dt.float32,
                            kind="ExternalInput").ap()
    w1_in = nc.dram_tensor("W1", [IN_CH, HID], dt.float32, kind="ExternalInput").ap()
    b1_in = nc.dram_tensor("b1", [HID, 1], dt.float32, kind="ExternalInput").ap()
    w2_in = nc.dram_tensor("W2", [HID, HID], dt.float32, kind="ExternalInput").ap()
    b2b_in = nc.dram_tensor("b2b", [128, HID], dt.float32, kind="ExternalInput").ap()
    wfc_in = nc.dram_tensor("Wfc", [HID, OUT], dt.float32, kind="ExternalInput").ap()
    bfc_in = nc.dram_tensor("bfc", [OUT, 1], dt.float32, kind="ExternalInput").ap()
    outp = nc.dram_tensor("outp", [OUT, G_PAD], dt.float32,
                          kind="ExternalOutput").ap()

    t1_local = nc.dram_tensor("t1_local", [NL, IN_CH], dt.bfloat16)
    T1 = nc.dram_tensor("T1", [NTOT, IN_CH], dt.bfloat16)
    t2_local = nc.dram_tensor("t2_local", [NL, HID], dt.bfloat16)
    T2 = nc.dram_tensor("T2", [NTOT, HID], dt.bfloat16)

    I128 = nc.inline_tensor(np.eye(128, dtype=np.float32), name="I128").ap()
    I64 = nc.inline_tensor(np.eye(HID, dtype=np.float32), name="I64").ap()

    rg = [list(range(NCORES))]

    with tile.TileContext(nc) as tc:
        with (
            tc.tile_pool(name="p", bufs=1) as pool,
            tc.tile_pool(name="pt", bufs=2) as tpool,
            tc.tile_pool(name="psA", bufs=2, space=bass.MemorySpace.PSUM) as psA,
            tc.tile_pool(name="psB", bufs=1, space=bass.MemorySpace.PSUM) as psB,
            tc.tile_pool(name="psP", bufs=1, space=bass.MemorySpace.PSUM) as psP,
        ):
            # ---- load inputs ----
            xp_t = pool.tile([128, COLS, IN_CH], dt.bfloat16, name="xp_t")
            nc.sync.dma_start(xp_t[:], xp_in[:])
            idx_t = pool.tile([128, S], dt.int32, name="idx_t")
            nc.sync.dma_start(idx_t[:], idx_in[:])
            dinv_t = pool.tile([128, COLS], dt.float32, name="dinv_t")
            nc.sync.dma_start(dinv_t[:], dinv_in[:])
            gid_t = pool.tile([128, COLS], dt.float32, name="gid_t")
            nc.sync.dma_start(gid_t[:], gid_in[:])
            cnt_t = pool.tile([OUT, G_PAD], dt.float32, name="cnt_t")
            nc.sync.dma_start(cnt_t[:], cnt_in[:])
            w1_t = pool.tile([IN_CH, HID], dt.float32, name="w1_t")
            nc.sync.dma_start(w1_t[:], w1_in[:])
            b1_t = pool.tile([HID, 1], dt.float32, name="b1_t")
            nc.sync.dma_start(b1_t[:], b1_in[:])
            w2_t = pool.tile([HID, HID], dt.float32, name="w2_t")
            nc.sync.dma_start(w2_t[:], w2_in[:])
            b2b_t = pool.tile([128, HID], dt.float32, name="b2b_t")
            nc.sync.dma_start(b2b_t[:], b2b_in[:])
            wfc_t = pool.tile([HID, OUT], dt.float32, name="wfc_t")
            nc.sync.dma_start(wfc_t[:], wfc_in[:])
            bfc_t = pool.tile([OUT, 1], dt.float32, name="bfc_t")
            nc.sync.dma_start(bfc_t[:], bfc_in[:])
            i128_t = pool.tile([128, 128], dt.float32, name="i128_t")
            nc.sync.dma_start(i128_t[:], I128[:])
            i64_t = pool.tile([HID, HID], dt.float32, name="i64_t")
            nc.sync.dma_start(i64_t[:], I64[:])

            iota_i = pool.tile([128, G_PAD], dt.int32, name="iota_i")
            nc.gpsimd.iota(iota_i[:], pattern=[[1, G_PAD]], base=0,
                           channel_multiplier=0)
            iota_f = pool.tile([128, G_PAD], dt.float32, name="iota_f")
            nc.vector.tensor_copy(iota_f[:], iota_i[:])

            # ---- t1 local block + AllGather ----
            nc.sync.dma_start(
                t1_local.ap().rearrange("(c p) f -> p c f", p=128), xp_t[:])
            nc.gpsimd.collective_compute(
                "AllGather", mybir.AluOpType.bypass, replica_groups=rg,
                ins=[t1_local.ap().opt()], outs=[T1.ap().opt()])

            # ---- L1 aggregation: per column K indirect adds + SBUF self ----
            agg1 = pool.tile([128, COLS, IN_CH], dt.float32, name="agg1")
            A1 = [pool.tile([128, IN_CH], dt.float32, name=f"A1_{c}")
                  for c in range(COLS)]
            off = 0
            for c in range(COLS):
                nc.vector.memset(A1[c][:], 0.0)
                for j in range(K_cols[c]):
                    nc.gpsimd.indirect_dma_start(
                        out=A1[c][:, :], out_offset=None, in_=T1.ap(),
                        in_offset=bass.IndirectOffsetOnAxis(
                            ap=idx_t[:, off + j:off + j + 1], axis=0),
                        compute_op=mybir.AluOpType.add)
                off += K_cols[c]
                nc.vector.tensor_add(A1[c][:], A1[c][:], xp_t[:, c, :])
                nc.scalar.mul(agg1[:, c, :], A1[c][:], dinv_t[:, c:c + 1])

            # ---- transpose agg1 -> rT [IN_CH, NL] ----
            rT = pool.tile([IN_CH, NL], dt.float32, name="rT")
            for c in range(COLS):
                pT = psA.tile([IN_CH, 128], dt.float32, name=f"pT{c}", tag="pT")
                nc.tensor.transpose(pT[:], agg1[:, c, :], i128_t[:])
                nc.scalar.copy(rT[:, c * 128:(c + 1) * 128], pT[:])

            # ---- per 512-node chunk: W1+relu+W2, transpose back, scale -> t2 ----
            t2self = pool.tile([128, COLS, HID], dt.bfloat16, name="t2self")
            for k in range(NCHUNK):
                sl = slice(k * 512, (k + 1) * 512)
                pW1 = psB.tile([HID, 512], dt.float32, name="pW1", tag="pW1")
                nc.tensor.matmul(pW1[:], w1_t[:], rT[:, sl])
                h1c = tpool.tile([HID, 512], dt.float32, name="h1c", tag="h1c")
                nc.scalar.activation(h1c[:], pW1[:], AF.Relu, bias=b1_t[:, 0:1])
                pW2 = psB.tile([HID, 512], dt.float32, name="pW2", tag="pW2")
                nc.tensor.matmul(pW2[:], w2_t[:], h1c[:])
                t2c = tpool.tile([HID, 512], dt.float32, name="t2c", tag="t2c")
                nc.scalar.copy(t2c[:], pW2[:])
                for q in range(4):
                    c = k * 4 + q
                    pR = psA.tile([128, HID], dt.float32, name=f"pR{c}", tag="pR")
                    nc.tensor.transpose(pR[:], t2c[:, q * 128:(q + 1) * 128], i64_t[:])
                    nc.scalar.mul(t2self[:, c, :], pR[:], dinv_t[:, c:c + 1])
                nc.sync.dma_start(
                    t2_local.ap()[k * 512:(k + 1) * 512, :]
                    .rearrange("(q p) f -> p q f", p=128),
                    t2self[:, k * 4:(k + 1) * 4, :])

            # ---- AllGather t2 ----
            nc.gpsimd.collective_compute(
                "AllGather", mybir.AluOpType.bypass, replica_groups=rg,
                ins=[t2_local.ap().opt()], outs=[T2.ap().opt()])

            # ---- L2 aggregation + h2 + pooling ----
            pPool = psP.tile([HID, G_PAD], dt.float32, name="pPool")
            off = 0
            for c in range(COLS):
                A2 = pool.tile([128, HID], dt.float32, name=f"A2_{c}")
                nc.vector.memset(A2[:], 0.0)
                for j in range(K_cols[c]):
                    nc.gpsimd.indirect_dma_start(
                        out=A2[:, :], out_offset=None, in_=T2.ap(),
                        in_offset=bass.IndirectOffsetOnAxis(
                            ap=idx_t[:, off + j:off + j + 1], axis=0),
                        compute_op=mybir.AluOpType.add)
                off += K_cols[c]
                nc.vector.tensor_add(A2[:], A2[:], t2self[:, c, :])
                h2a = tpool.tile([128, HID], dt.float32, name="h2a", tag="h2a")
                nc.scalar.mul(h2a[:], A2[:], dinv_t[:, c:c + 1])
                h2b = tpool.tile([128, HID], dt.float32, name="h2b", tag="h2b")
                nc.vector.tensor_add(h2b[:], h2a[:], b2b_t[:])
                h2col = tpool.tile([128, HID], dt.bfloat16, name="h2col", tag="h2col")
                nc.scalar.activation(h2col[:], h2b[:], AF.Relu)
                Pcol = tpool.tile([128, G_PAD], dt.bfloat16, name="Pcol", tag="Pcol")
                nc.vector.tensor_scalar(
                    Pcol[:], iota_f[:], gid_t[:, c:c + 1], None,
                    op0=mybir.AluOpType.is_equal)
                nc.tensor.matmul(pPool[:], h2col[:], Pcol[:],
                                 start=(c == 0), stop=(c == COLS - 1))

            # ---- FC + sigmoid ----
            pooledT = pool.tile([HID, G_PAD], dt.float32, name="pooledT")
            nc.scalar.copy(pooledT[:], pPool[:])
            pF = psB.tile([OUT, G_PAD], dt.float32, name="pF", tag="pF")
            nc.tensor.matmul(pF[:], wfc_t[:], pooledT[:])
            gsc = pool.tile([OUT, G_PAD], dt.float32, name="gsc")
            nc.vector.tensor_mul(gsc[:], pF[:], cnt_t[:])
            res = pool.tile([OUT, G_PAD], dt.float32, name="res")
            nc.scalar.activation(res[:], gsc[:], AF.Sigmoid, bias=bfc_t[:, 0:1])
            nc.sync.dma_start(outp[:], res[:])

    nc.compile()
    return nc


def kernel(x, edge_index, batch, W1, b1, W2, b2, Wfc, bfc):
    import os
    x = np.asarray(x, np.float32)
    src = np.asarray(edge_index[0]).astype(np.int64)
    dst = np.asarray(edge_index[1]).astype(np.int64)
    batch = np.asarray(batch).astype(np.int64)
    W1 = np.asarray(W1, np.float32); b1 = np.asarray(b1, np.float32)
    W2 = np.asarray(W2, np.float32); b2 = np.asarray(b2, np.float32)
    Wfc = np.asarray(Wfc, np.float32); bfc = np.asarray(bfc, np.float32)

    # ---------- host index preprocessing (vectorized) ----------
    deg = np.bincount(dst, minlength=N_NODES).astype(np.float32) + 1.0
    dinv = 1.0 / np.sqrt(deg)

    order = np.argsort(dst.astype(np.int32), kind="stable")
    dst_s = dst[order]; src_s = src[order]
    starts = np.searchsorted(dst_s, np.arange(N_NODES + 1))

    gcnt = np.bincount(batch, minlength=N_GRAPHS)
    gcum = np.concatenate([[0], np.cumsum(gcnt)])
    bounds = [0]
    gb = [0]
    for d in range(1, NCORES):
        tgt = d * (N_NODES // NCORES)
        g = int(np.argmin(np.abs(gcum - tgt)))
        bounds.append(int(gcum[g])); gb.append(g)
    bounds.append(N_NODES); gb.append(N_GRAPHS)

    ldeg_all = (starts[1:] - starts[:-1]).astype(np.int64)

    rank_of_node = np.empty(N_NODES, np.int64)   # node -> rank within owner
    owner_of_node = np.empty(N_NODES, np.int64)
    gn_of_rank = []                              # per core: rank -> node (-1 pad)
    colmax = np.zeros((NCORES, COLS), np.int64)
    for d in range(NCORES):
        s_d, e_d = bounds[d], bounds[d + 1]
        nloc = e_d - s_d
        assert nloc < NL, (nloc, NL)
        r2l = np.argsort(-ldeg_all[s_d:e_d], kind="stable")
        rank_of_node[s_d + r2l] = np.arange(nloc)
        owner_of_node[s_d:e_d] = d
        gn = np.full(NL, -1, np.int64)
        gn[:nloc] = s_d + r2l
        gn_of_rank.append(gn)
        rd = np.zeros(NL, np.int64)
        rd[:nloc] = ldeg_all[s_d:e_d][r2l]
        colmax[d] = rd.reshape(COLS, 128).max(axis=1)
    K_cols = tuple(int(v) for v in colmax.max(axis=0))
    S = sum(K_cols)
    soff = np.zeros(COLS, np.int64)              # slot offset per column
    acc = 0
    for c in range(COLS):
        soff[c] = acc
        acc += K_cols[c]

    tablerow = owner_of_node * NL + rank_of_node     # node -> table row
    nlocs = np.array([bounds[d + 1] - bounds[d] for d in range(NCORES)])
    dmin = int(np.argmin(nlocs))
    PADROW = dmin * NL + NL - 1

    # per-core inputs
    in_maps = []
    for d in range(NCORES):
        s_d, e_d = bounds[d], bounds[d + 1]
        nloc = e_d - s_d
        gn = gn_of_rank[d]
        ok = gn >= 0

        ia = np.full((128, S), PADROW, np.int32)
        rr = np.arange(NL)
        pp = rr % 128; cc = rr // 128
        # neighbor slots (edge-driven, vectorized)
        e0, e1 = starts[s_d], starts[e_d]
        dst_e = dst_s[e0:e1]
        src_e = src_s[e0:e1]
        j_e = np.arange(e0, e1) - starts[dst_e]
        r_e = rank_of_node[dst_e]
        p_e = r_e % 128; c_e = r_e // 128
        ia[p_e, soff[c_e] + j_e] = tablerow[src_e].astype(np.int32)

        xp = np.zeros((128, COLS, IN_CH), np.float32)
        xv = x[gn[ok]] * dinv[gn[ok]][:, None]
        xp[pp[ok], cc[ok]] = xv

        dv = np.zeros((128, COLS), np.float32)
        dv[pp[ok], cc[ok]] = dinv[gn[ok]]

        ng = gb[d + 1] - gb[d]
        assert ng < G_PAD, ng
        gi = np.full((128, COLS), G_PAD - 1, np.float32)
        gi[pp[ok], cc[ok]] = (batch[gn[ok]] - gb[d]).astype(np.float32)

        ci = np.ones((OUT, G_PAD), np.float32)
        ci[:, :ng] = 1.0 / np.maximum(gcnt[gb[d]:gb[d + 1]], 1.0)[None, :]

        in_maps.append({
            "xp": xp.astype(ml_dtypes.bfloat16),
            "idx": ia,
            "dinv": dv,
            "gid": gi,
            "cntinv": ci,
            "W1": W1, "b1": b1.reshape(HID, 1), "W2": W2,
            "b2b": np.broadcast_to(b2, (128, HID)).copy(),
            "Wfc": Wfc, "bfc": bfc.reshape(OUT, 1),
        })

    # ---------- build + run ----------
    key = ("V2", K_cols)
    if key not in _cache:
        _cache[key] = _build(list(K_cols))
    nc = _cache[key]

    from concourse.bass_utils import run_bass_kernel_spmd
    trace = bool(os.environ.get("BASS_TRACE"))
    t0 = time.perf_counter()
    res = run_bass_kernel_spmd(nc, in_maps, core_ids=list(range(NCORES)),
                               trace=trace)
    LAST_EXEC_WALLS.append(time.perf_counter() - t0)
    if res.exec_time_ns is not None:
        LAST_EXEC_NS.append(res.exec_time_ns)

    out = np.zeros((N_GRAPHS, OUT), np.float32)
    for d in range(NCORES):
        ng = gb[d + 1] - gb[d]
        out[gb[d]:gb[d + 1]] = np.asarray(
            res.results[d]["outp"], np.float32)[:, :ng].T
    return out
